# revision 1
# baseline (speedup 1.0000x reference)
"""Trainium2 Bass kernel for the FFTBlock problem (B=2, C=32, H=2688, W=128).

Math (reference):
  spatial  = relu(conv7x1_s7(x) + b_spatial)                        [B,C,384,W]
  spectral = irfft(relu(w_spectral @ rfft_concat(x) + b_spectral))  per (b,c,w)
  out = spatial + spectral

Transformation: rfft/irfft along H are linear, so with F the real-ified rfft
matrix (2 dead rows dropped -> [2688, 2688]) and G the irfft matrix (2 dead
cols dropped -> [384, 384]):
  spectral_col = G @ relu(A @ x_col + b),   A = w_spectral @ F  [384, 2688]

A is weight-only, so it is folded on the HOST (numpy GEMM); the device runs a
single launch per core (W sharded 8 x 16 columns):
  GEMM1  conv[384, 1024] = A @ x_cols   (f16, 21 k-tiles, 6 PSUM banks)
  relu   (ACT, bias)            -> f16
  GEMM2  spec[384, 1024] = G @ relu     (f16, 3 k-tiles)
  spatial conv as fp8-e4m3 DoubleRow GEMM [112,2,32]^T @ [112,2,12288]
         (w_spatial scaled x32 on host, un-scaled in the relu activation;
          fp8 error ~7e-3 rel, well inside the 2e-2 gate)
  outputs spec_out/spat_out written separately in f16; host adds + unshards.

All DRAM layouts are pre-swizzled on host to partition-major [128, k, n] so
every DMA moves long contiguous runs (>= 512B descriptors, full bandwidth).
"""

import os

import numpy as np
import ml_dtypes

import concourse.bacc as bacc
import concourse.mybir as mybir
import concourse.tile as tile
from concourse.bass_utils import run_bass_kernel_spmd
from concourse.alu_op_type import AluOpType

N_CORES = 8
B, C, H, W = 2, 32, 2688, 128
FREQ_IN = H // 2 + 1            # 1345
OUT_H = 384
FREQ_OUT = OUT_H // 2 + 1       # 193
MO = 2 * FREQ_OUT - 2           # 384 usable conv channels
WS = W // N_CORES               # 16 width columns per core
NCOL = B * C * WS               # 1024 spectral columns per core
NSP = B * OUT_H * WS            # 12288 spatial columns per core
KSP = C * 7                     # 224 spatial reduction
KH = KSP // 2                   # 112 partitions for DoubleRow spatial

KT1 = H // 128                  # 21 k-tiles for GEMM1
MT1 = MO // 128                 # 3 m-tiles
NT = NCOL // 512                # 2 n-tiles
MT2 = OUT_H // 128              # 3 m-tiles for GEMM2
NSPC = NSP // 512               # 24 spatial chunks
SP_GRP = NSPC // 4              # 6 chunks per xsp load group
GW = SP_GRP * 512               # 3072 cols per xsp group

WSP_SCALE = 32.0                # fp8 range helper for the tiny spatial weights

F32 = mybir.dt.float32
F16 = mybir.dt.float16
F8E4 = mybir.dt.float8e4
F8E3 = mybir.dt.float8e3
RELU = mybir.ActivationFunctionType.Relu
DR = mybir.MatmulPerfMode.DoubleRow
E4M3 = ml_dtypes.float8_e4m3
E3M4 = ml_dtypes.float8_e3m4

_cache = {}
LAST_EXEC_NS = None
LAST_FOLD_NS = None


def _dft_constants():
    """F [2688, 2688] (rfft, ortho, dead rows dropped) and G [384, 384]
    (irfft, ortho, dead cols dropped)."""
    if "F" in _cache:
        return _cache["F"], _cache["G"]
    Fc = np.fft.rfft(np.eye(H), axis=0, norm="ortho")       # [1345, 2688]
    F = np.concatenate([Fc.real, Fc.imag[1:FREQ_IN - 1]], axis=0)
    F = np.ascontiguousarray(F, dtype=np.float32)           # [2688, 2688]
    G_re = np.fft.irfft(np.eye(FREQ_OUT), n=OUT_H, axis=0, norm="ortho")
    G_im = np.fft.irfft(1j * np.eye(FREQ_OUT), n=OUT_H, axis=0, norm="ortho")
    G = np.concatenate([G_re, G_im[:, 1:FREQ_OUT - 1]], axis=1)
    G = np.ascontiguousarray(G, dtype=np.float32)           # [384, 384]
    _cache["F"] = F
    _cache["G"] = G
    return F, G


def _spec_keep_idx():
    keep_f = list(range(FREQ_IN)) + [FREQ_IN + k for k in range(1, FREQ_IN - 1)]
    keep_o = list(range(FREQ_OUT)) + [FREQ_OUT + k for k in range(1, FREQ_OUT - 1)]
    return np.array(keep_f), np.array(keep_o)


def _build_main():
    if "main" in _cache:
        return _cache["main"]
    nc = bacc.Bacc("TRN2", target_bir_lowering=False, debug=False,
                   num_devices=N_CORES)
    at = nc.dram_tensor("at", [128, KT1 * MO], F16, kind="ExternalInput").ap()
    xt = nc.dram_tensor("xt", [128, KT1 * NCOL], F8E3,
                        kind="ExternalInput").ap()
    gt = nc.dram_tensor("gt", [128, MT2 * OUT_H], F16, kind="ExternalInput").ap()
    bspec = nc.dram_tensor("bspec", [128, MT1], F32, kind="ExternalInput").ap()
    wsp = nc.dram_tensor("wsp", [KH, 2 * C], F8E4, kind="ExternalInput").ap()
    bsp = nc.dram_tensor("bsp", [C, 1], F32, kind="ExternalInput").ap()
    xsp = nc.dram_tensor("xsp", [KH, 2 * NSP], F8E4, kind="ExternalInput").ap()
    spec_out = nc.dram_tensor("spec_out", [128, MT2 * NCOL], F16,
                              kind="ExternalOutput").ap()
    spat_out = nc.dram_tensor("spat_out", [C, NSP], F16,
                              kind="ExternalOutput").ap()

    with tile.TileContext(nc) as tc:
        with tc.tile_pool(name="const", bufs=1) as cst, \
             tc.tile_pool(name="atp", bufs=1) as atp, \
             tc.tile_pool(name="xtp", bufs=1) as xtp, \
             tc.tile_pool(name="xspp", bufs=1) as xspp, \
             tc.tile_pool(name="relu", bufs=1) as rlp, \
             tc.tile_pool(name="spst", bufs=2) as spst, \
             tc.tile_pool(name="outp", bufs=1) as outp, \
             tc.tile_pool(name="ps", bufs=1, space="PSUM") as psp, \
             tc.tile_pool(name="psw", bufs=1, space="PSUM") as psw:

            # ---------------- SBUF tiles ----------------
            # GEMM1 reads f16 stationary A against e3m4 moving x (mixed-dtype
            # matmul verified exact on HW).  e3m4 xt halves the dominant load
            # stream, leaving the DMA comfortably ahead of the PE everywhere;
            # any PE stall would reset the p-state ramp (~1.5us penalty).
            KGRP = [(0, 1), (1, 3), (3, 6), (6, 9), (9, 12), (12, 15),
                    (15, 18), (18, 21)]
            at_g, xt_g, at_t, xt_t = [], [], [], []
            for gi, (k0, k1) in enumerate(KGRP):
                ag = atp.tile([128, (k1 - k0) * MO], F16, tag=f"at{gi}",
                              name=f"at{gi}")
                xg = xtp.tile([128, (k1 - k0) * NCOL], F8E3, tag=f"xt{gi}",
                              name=f"xt{gi}")
                at_g.append(ag)
                xt_g.append(xg)
                for j in range(k1 - k0):
                    at_t.append(ag[:, MO * j:MO * (j + 1)])
                    xt_t.append(xg[:, NCOL * j:NCOL * (j + 1)])
            gt_sb = cst.tile([128, MT2 * OUT_H], F16, tag="gt", name="gt")
            gt_t = [gt_sb[:, OUT_H * k:OUT_H * (k + 1)] for k in range(MT2)]
            bspec_sb = cst.tile([128, MT1], F32, tag="bspec", name="bspec")
            wsp_sb = cst.tile([KH, 2 * C], F8E4, tag="wsp", name="wsp")
            bsp_sb = cst.tile([C, 1], F32, tag="bsp", name="bsp")
            xsp_g = [xspp.tile([KH, 2 * GW], F8E4, tag=f"xsp{g}",
                               name=f"xsp{g}") for g in range(4)]
            relu_t = [rlp.tile([128, NCOL], F16, tag=f"relu{m}",
                               name=f"relu{m}") for m in range(MT1)]
            so_t = [outp.tile([128, NCOL], F16, tag=f"so{m2}", name=f"so{m2}")
                    for m2 in range(MT2)]

            # ---------------- DMA emission (sync queue, in order) --------
            def load_kg(gi):
                k0, k1 = KGRP[gi]
                nc.sync.dma_start(at_g[gi][:], at[:, MO * k0:MO * k1])
                nc.sync.dma_start(xt_g[gi][:], xt[:, NCOL * k0:NCOL * k1])

            def load_xsp(g):
                src = xsp.rearrange("p (i n) -> p i n", i=2)[:, :,
                                                            GW * g:GW * (g + 1)]
                dst = xsp_g[g][:].rearrange("p (i n) -> p i n", i=2)
                nc.sync.dma_start(dst, src)

            load_kg(0)
            load_kg(1)
            load_kg(2)
            load_kg(3)
            nc.sync.dma_start(wsp_sb[:], wsp[:])
            nc.sync.dma_start(bsp_sb[:], bsp[:])
            load_xsp(0)
            nc.sync.dma_start(bspec_sb[:], bspec[:])
            nc.sync.dma_start(gt_sb[:], gt[:])
            load_kg(4)
            load_xsp(1)
            load_kg(5)
            load_xsp(2)
            load_kg(6)
            load_xsp(3)
            load_kg(7)

            # ---------------- compute ----------------
            ps_mn = {(m, n): psp.tile([128, 512], F32, tag=f"g1m{m}n{n}",
                                      name=f"g1m{m}n{n}")
                     for m in range(MT1) for n in range(NT)}
            wsp_v = wsp_sb[:].rearrange("p (i m) -> p i m", i=2)

            def g1_step(k):
                for m in range(MT1):
                    msl = slice(128 * m, 128 * (m + 1))
                    for n in range(NT):
                        nc.tensor.matmul(ps_mn[(m, n)][:], at_t[k][:, msl],
                                         xt_t[k][:, 512 * n:512 * (n + 1)],
                                         start=(k == 0), stop=(k == KT1 - 1))

            sp_tiles = {}

            def sp_chunk(j):
                # spatial relu alternates ACT / DVE so neither engine's queue
                # falls behind the 2-bank PSUM rotation.  Both write
                # 32*relu(conv+b) (weights are pre-scaled x32; DVE has no
                # scale operand) -- the host multiplies spat_out by 1/32.
                g, jj = j // SP_GRP, j % SP_GRP
                if g not in sp_tiles:
                    sp_tiles[g] = spst.tile([C, GW], F16, tag=f"sp{g % 2}",
                                            name=f"sp{g}")
                xv = xsp_g[g][:].rearrange("p (i n) -> p i n", i=2)
                ps = psw.tile([C, 512], F32, tag=f"spp{j % 2}", name=f"spp{j}")
                nc.tensor.matmul(ps[:], wsp_v,
                                 xv[:, :, 512 * jj:512 * (jj + 1)],
                                 start=True, stop=True, perf_mode=DR)
                dst = sp_tiles[g][:, 512 * jj:512 * (jj + 1)]
                if j % 2 == 0:
                    nc.scalar.activation(dst, ps[:], RELU, bias=bsp_sb[:])
                else:
                    nc.vector.tensor_scalar(dst, ps[:], bsp_sb[:], 0.0,
                                            AluOpType.add, AluOpType.max)
                if jj == SP_GRP - 1:
                    nc.gpsimd.dma_start(spat_out[:, GW * g:GW * (g + 1)],
                                        sp_tiles[g][:])

            def relu_mn(m, n):
                # alternate relu between ACT and DVE so GEMM2's inputs are
                # ready ~2x sooner after the k20 stop
                nsl = slice(512 * n, 512 * (n + 1))
                if m == 1:
                    nc.vector.tensor_scalar(relu_t[m][:, nsl], ps_mn[(m, n)][:],
                                            bspec_sb[:, m:m + 1], 0.0,
                                            AluOpType.add, AluOpType.max)
                else:
                    nc.scalar.activation(relu_t[m][:, nsl], ps_mn[(m, n)][:],
                                         RELU, bias=bspec_sb[:, m:m + 1])

            # GEMM2 runs k-major so its first matmul waits only on
            # relu(m=0, n) instead of all three relus of that n-half
            ps2_m = {}

            def g2_n(n):
                nsl = slice(512 * n, 512 * (n + 1))
                for m2 in range(MT2):
                    ps2_m[(m2, n)] = psp.tile([128, 512], F32,
                                              tag=f"g1m{m2}n{n}", name="g2")
                for k in range(MT2):
                    for m2 in range(MT2):
                        nc.tensor.matmul(ps2_m[(m2, n)][:],
                                         gt_t[k][:, 128 * m2:128 * (m2 + 1)],
                                         relu_t[k][:, nsl],
                                         start=(k == 0), stop=(k == MT2 - 1))

            COPY = mybir.ActivationFunctionType.Copy

            def g2_out(n):
                nsl = slice(512 * n, 512 * (n + 1))
                for m2 in range(MT2):
                    # copies alternate DVE / ACT so the three tail copies
                    # drain in parallel instead of serializing on DVE
                    if m2 == 1:
                        nc.scalar.activation(so_t[m2][:, nsl],
                                             ps2_m[(m2, n)][:], COPY)
                    else:
                        nc.vector.tensor_copy(so_t[m2][:, nsl],
                                              ps2_m[(m2, n)][:])
                    # spec stores alternate sync / Pool queues so the three
                    # tail stores drain in parallel instead of serializing
                    eng = nc.gpsimd if m2 == 1 else nc.sync
                    eng.dma_start(
                        spec_out[:, NCOL * m2 + 512 * n:
                                 NCOL * m2 + 512 * (n + 1)],
                        so_t[m2][:, nsl])

            # PE order: GEMM1 k-steps with 20 of the 24 spatial chunks spread
            # over them; 4 chunks stay back as PE filler during the relu
            # latency window so the PE p-state never drops before GEMM2.
            # all 24 spatial chunks ride k-steps: 1/step k6-k11 (xsp0 lands
            # ~10.6us, PE reaches k6 ~11.9us), 2/step k12-k20; the tail is
            # then pure relu+GEMM2 with no ACT-queue backlog
            sp_j = iter(range(NSPC))
            for k in range(KT1):
                if 6 <= k < 12:
                    sp_chunk(next(sp_j))
                elif k >= 12:
                    sp_chunk(next(sp_j))
                    sp_chunk(next(sp_j))
                g1_step(k)
            for m in range(MT1):
                relu_mn(m, 0)
            g2_n(0)
            for m in range(MT1):
                relu_mn(m, 1)
            g2_out(0)
            g2_n(1)
            g2_out(1)

    nc.compile()
    _cache["main"] = nc
    return nc


def kernel(x, w_spatial, b_spatial, w_spectral, b_spectral):
    x = np.ascontiguousarray(x, dtype=np.float32)
    w_spatial = np.asarray(w_spatial, dtype=np.float32)
    b_spatial = np.asarray(b_spatial, dtype=np.float32)
    w_spectral = np.asarray(w_spectral, dtype=np.float32)
    b_spectral = np.asarray(b_spectral, dtype=np.float32)

    F, G = _dft_constants()
    keep_f, keep_o = _spec_keep_idx()
    core_ids = list(range(N_CORES))

    # ---- host fold: A = W_spec @ F  (weight-only preprocessing) ----
    A = w_spectral[keep_o][:, keep_f] @ F                    # [384, 2688]
    at_np = np.ascontiguousarray(
        A.reshape(MO, KT1, 128).transpose(2, 1, 0).reshape(128, KT1 * MO)
    ).astype(np.float16)
    gt_np = np.ascontiguousarray(
        G.T.reshape(MT2, 128, OUT_H).transpose(1, 0, 2)
        .reshape(128, MT2 * OUT_H)).astype(np.float16)
    bspec_np = np.ascontiguousarray(
        b_spectral[keep_o].reshape(MT1, 128).T).astype(np.float32)
    wsp_np = np.ascontiguousarray(
        (w_spatial[:, :, :, 0].transpose(1, 2, 0).reshape(KSP, C) * WSP_SCALE)
        .reshape(2, KH, C).transpose(1, 0, 2).reshape(KH, 2 * C)
    ).astype(E4M3)
    # device computes 32*relu(conv+b) (weights x32, bias x32); host divides
    bsp_np = np.ascontiguousarray(
        b_spatial.reshape(C, 1) * WSP_SCALE).astype(np.float32)

    in_maps = []
    for i in core_ids:
        xs = x[:, :, :, WS * i:WS * (i + 1)]                 # [B, C, H, WS]
        xt_np = np.ascontiguousarray(
            xs.transpose(2, 0, 1, 3).reshape(KT1, 128, NCOL)
            .transpose(1, 0, 2).reshape(128, KT1 * NCOL)).astype(E3M4)
        xsp_np = np.ascontiguousarray(
            xs.reshape(B, C, OUT_H, 7, WS).transpose(1, 3, 0, 2, 4)
            .reshape(2, KH, NSP).transpose(1, 0, 2).reshape(KH, 2 * NSP)
        ).astype(E4M3)
        in_maps.append({"at": at_np, "xt": xt_np, "gt": gt_np,
                        "bspec": bspec_np, "wsp": wsp_np, "bsp": bsp_np,
                        "xsp": xsp_np})

    nc = _build_main()
    kw = {}
    if bool(int(os.environ.get("KERNEL_TRACE", "0"))):
        d = os.environ.get("KERNEL_TRACE_DIR", "/tmp/ktrace") + "/main"
        os.makedirs(d, exist_ok=True)
        kw = dict(trace=True, tmpdir=d)
    res = run_bass_kernel_spmd(nc, in_maps, core_ids, **kw)
    global LAST_EXEC_NS
    LAST_EXEC_NS = res.exec_time_ns

    # ---- host: add branches + unshard ----
    out = np.empty((B, C, OUT_H, W), np.float32)
    for i in core_ids:
        spec = (res.results[i]["spec_out"].astype(np.float32)
                .reshape(128, MT2, B, C, WS).transpose(2, 3, 1, 0, 4)
                .reshape(B, C, OUT_H, WS))
        spat = (res.results[i]["spat_out"].astype(np.float32)
                .reshape(C, B, OUT_H, WS).transpose(1, 0, 2, 3))
        out[:, :, :, WS * i:WS * (i + 1)] = spec + spat * (1.0 / WSP_SCALE)
    return out



# revision 10
# speedup vs baseline: 1.3008x; 1.3008x over previous
"""Trainium2 Bass kernel for the FFTBlock problem (B=2, C=32, H=2688, W=128).

Math (reference):
  spatial  = relu(conv7x1_s7(x) + b_spatial)                        [B,C,384,W]
  spectral = irfft(relu(w_spectral @ rfft_concat(x) + b_spectral))  per (b,c,w)
  out = spatial + spectral

rfft/irfft along H are linear, so with F the real-ified rfft matrix and G the
irfft matrix (dead rows/cols dropped):
  spectral_col = G @ relu(A @ x_col + b),   A = w_spectral @ F  [384, 2688]

Device plan (W sharded 8 x 16 columns, one launch per core):
  GEMM1  conv[384, 1024] = A @ x_cols:
         - KD_DR DoubleRow steps (K=256, both operands e4m3, x64 A scale)
         - NN normal-mode k-tiles (K=128, f16 A x e3m4 x) for accuracy;
           they double as fat PE fillers mid-loop so the p-state never drops
  relu   (ACT/DVE/Pool, bias)            -> f16 (scaled x64; G absorbs /64)
  GEMM2  spec[384, 1024] = (G/64) @ relu  (f16, 3 k-tiles)
  spatial conv TRANSPOSED: stationary = x chunks [(c,t)=224+bias row,
         h'-block], moving = w_spatial (e4m3 x32, bias folded as an extra
         contraction row), DoubleRow -> psum [h', (b,w,co)] -- same layout as
         spec, so the final add happens on-device and only ONE output tensor
         is stored.
  out[m2,n] = ps2 + sp_relu  (DVE/Pool scalar_tensor_tensor), stored f16.

Column order everywhere is (b, w, c) so the spatial conv's 32-channel output
blocks are contiguous in the spectral column space. All DRAM layouts are
pre-swizzled on host to partition-major so every DMA moves >=512B runs.
"""

import os

import numpy as np
import ml_dtypes

import concourse.bacc as bacc
import concourse.mybir as mybir
import concourse.tile as tile
from concourse.bass_utils import run_bass_kernel_spmd
from concourse.alu_op_type import AluOpType

N_CORES = 8
B, C, H, W = 2, 32, 2688, 128
FREQ_IN = H // 2 + 1            # 1345
OUT_H = 384
FREQ_OUT = OUT_H // 2 + 1       # 193
MO = 2 * FREQ_OUT - 2           # 384 usable conv channels
WS = W // N_CORES               # 16 width columns per core
NCOL = B * WS * C               # 1024 spectral columns per core, (b, w, c)
NSP = B * OUT_H * WS            # 12288 spatial cols (b, h', w)
OLDK = H // 128                 # 21 k-tiles of 128
KD_DR = 9                       # DoubleRow steps (cover old-k 0..2*KD_DR-1)
NN = OLDK - 2 * KD_DR           # normal-mode k-tiles (incl. the odd tail)
MT = 3                          # 128-row m-tiles (G1 out / G2 out)
NT = 2                          # 512-col n halves; n == b
KH = 112                        # (c,t) DR half-pairs for spatial
KHB = KH + 1                    # +1 bias row

AT_SCALE = 64.0                 # fp8 range helper for A = w_spec @ F
WSP_SCALE = 32.0                # fp8 range helper for the tiny spatial weights

F32 = mybir.dt.float32
F16 = mybir.dt.float16
F8E4 = mybir.dt.float8e4
F8E3 = mybir.dt.float8e3
RELU = mybir.ActivationFunctionType.Relu
DR = mybir.MatmulPerfMode.DoubleRow
E4M3 = ml_dtypes.float8_e4m3
E3M4 = ml_dtypes.float8_e3m4

_cache = {}
LAST_EXEC_NS = None


def _dft_constants():
    """F [2688, 2688] (rfft, ortho, dead rows dropped) and G [384, 384]
    (irfft, ortho, dead cols dropped)."""
    if "F" in _cache:
        return _cache["F"], _cache["G"]
    Fc = np.fft.rfft(np.eye(H), axis=0, norm="ortho")       # [1345, 2688]
    F = np.concatenate([Fc.real, Fc.imag[1:FREQ_IN - 1]], axis=0)
    F = np.ascontiguousarray(F, dtype=np.float32)           # [2688, 2688]
    G_re = np.fft.irfft(np.eye(FREQ_OUT), n=OUT_H, axis=0, norm="ortho")
    G_im = np.fft.irfft(1j * np.eye(FREQ_OUT), n=OUT_H, axis=0, norm="ortho")
    G = np.concatenate([G_re, G_im[:, 1:FREQ_OUT - 1]], axis=1)
    G = np.ascontiguousarray(G, dtype=np.float32)           # [384, 384]
    _cache["F"] = F
    _cache["G"] = G
    return F, G


def _spec_keep_idx():
    keep_f = list(range(FREQ_IN)) + [FREQ_IN + k for k in range(1, FREQ_IN - 1)]
    keep_o = list(range(FREQ_OUT)) + [FREQ_OUT + k for k in range(1, FREQ_OUT - 1)]
    return np.array(keep_f), np.array(keep_o)


def _g1_order():
    """PE-side k-step order: DR steps with the fat normal tiles interleaved
    early-mid so the PE tracks the DMA stream without stalling."""
    order = [("d", s) for s in range(KD_DR)]
    pos = 2
    for j in range(NN):
        order.insert(min(pos, len(order)), ("n", j))
        pos += 2
    return order


def _build_main():
    if "main" in _cache:
        return _cache["main"]
    nc = bacc.Bacc("TRN2", target_bir_lowering=False, debug=False,
                   num_devices=N_CORES)
    at = nc.dram_tensor("at", [128, KD_DR * 2 * MO], F8E4,
                        kind="ExternalInput").ap()
    an = nc.dram_tensor("an", [128, NN * MO], F16, kind="ExternalInput").ap()
    xt = nc.dram_tensor("xt", [128, KD_DR * 2 * NCOL], F8E4,
                        kind="ExternalInput").ap()
    xn = nc.dram_tensor("xn", [128, NN * NCOL], F8E3,
                        kind="ExternalInput").ap()
    gt = nc.dram_tensor("gt", [128, MT * MO], F16, kind="ExternalInput").ap()
    bspec = nc.dram_tensor("bspec", [128, MT], F32, kind="ExternalInput").ap()
    wsp = nc.dram_tensor("wsp", [KHB, 2 * C], F8E4, kind="ExternalInput").ap()
    xsp = nc.dram_tensor("xsp", [KHB, 2 * NSP], F8E4,
                         kind="ExternalInput").ap()
    eye = nc.dram_tensor("eye", [128, 128], F16, kind="ExternalInput").ap()
    out_d = nc.dram_tensor("out", [128, MT * NCOL], F16,
                           kind="ExternalOutput").ap()

    with tile.TileContext(nc) as tc:
        with tc.tile_pool(name="const", bufs=1) as cst, \
             tc.tile_pool(name="atp", bufs=1) as atp, \
             tc.tile_pool(name="xtp", bufs=1) as xtp, \
             tc.tile_pool(name="xspp", bufs=1) as xspp, \
             tc.tile_pool(name="relu", bufs=1) as rlp, \
             tc.tile_pool(name="outp", bufs=1) as outp, \
             tc.tile_pool(name="ps", bufs=1, space="PSUM") as psp, \
             tc.tile_pool(name="psw", bufs=1, space="PSUM") as psw:

            # ---------------- SBUF tiles ----------------
            AGRP = [(0, 3), (3, 6), (6, KD_DR)]
            at_g = {}
            for g0, g1 in AGRP:
                at_g[g0] = atp.tile([128, (g1 - g0) * 2 * MO], F8E4,
                                    tag=f"at{g0}", name=f"at{g0}")
            an_sb = atp.tile([128, NN * MO], F16, tag="an", name="an")
            xt_t = [xtp.tile([128, 2 * NCOL], F8E4, tag=f"xt{s}",
                             name=f"xt{s}") for s in range(KD_DR)]
            xn_t = [xtp.tile([128, NCOL], F8E3, tag=f"xn{j}",
                             name=f"xn{j}") for j in range(NN)]
            gt_sb = cst.tile([128, MT * MO], F16, tag="gt", name="gt")
            eye_sb = cst.tile([128, 128], F16, tag="eye", name="eye")
            bspec_sb = cst.tile([128, MT], F32, tag="bspec", name="bspec")
            wsp_sb = cst.tile([KHB, 2 * C], F8E4, tag="wsp", name="wsp")
            xsp_g = {(b, m2): xspp.tile([KHB, 2 * 2048], F8E4,
                                        tag=f"xsp{b}{m2}", name=f"xsp{b}{m2}")
                     for b in range(B) for m2 in range(MT)}
            relu_t = [rlp.tile([128, NCOL], F16, tag=f"relu{m}",
                               name=f"relu{m}") for m in range(MT)]
            sp_sb = [rlp.tile([128, NCOL], F16, tag=f"sp{m2}",
                              name=f"sp{m2}") for m2 in range(MT)]
            out_sb = [outp.tile([128, NCOL], F16, tag=f"o{m2}",
                                name=f"o{m2}") for m2 in range(MT)]

            def at_s(s, m):
                """DR stationary [128, 2, 128] for DR step s, m-tile m."""
                g0 = max(g for g, _ in AGRP if g <= s)
                off = (s - g0) * 2 * MO
                v = at_g[g0][:, off:off + 2 * MO].rearrange(
                    "p (i m) -> p i m", i=2)
                return v[:, :, 128 * m:128 * (m + 1)]

            def xt_s(s, n):
                v = xt_t[s][:].rearrange("p (i n) -> p i n", i=2)
                return v[:, :, 512 * n:512 * (n + 1)]

            wsp_v = wsp_sb[:].rearrange("p (i m) -> p i m", i=2)
            gt_km = lambda k, m2: gt_sb[:, k * MO + 128 * m2:
                                        k * MO + 128 * (m2 + 1)]

            # ---------------- DMA emission (sync queue, in order) --------
            def load_at(g0):
                g1 = dict(AGRP)[g0]
                nc.sync.dma_start(at_g[g0][:],
                                  at[:, g0 * 2 * MO:g1 * 2 * MO])

            def load_xt(s):
                nc.sync.dma_start(xt_t[s][:],
                                  xt[:, s * 2 * NCOL:(s + 1) * 2 * NCOL])

            def load_xn(j):
                nc.sync.dma_start(xn_t[j][:],
                                  xn[:, j * NCOL:(j + 1) * NCOL])

            def load_xsp(b, m2):
                src = xsp.rearrange("p (i n) -> p i n", i=2)[
                    :, :, b * (OUT_H * WS) + m2 * 2048:
                    b * (OUT_H * WS) + (m2 + 1) * 2048]
                dst = xsp_g[(b, m2)][:].rearrange("p (i n) -> p i n", i=2)
                nc.sync.dma_start(dst, src)

            # order tuned so the PE (starting at xt0) never starves and the
            # last input (xsp b1 m2=2) has the shortest dependent chain
            nc.sync.dma_start(gt_sb[:], gt[:])
            nc.sync.dma_start(bspec_sb[:], bspec[:])
            nc.sync.dma_start(wsp_sb[:], wsp[:])
            nc.sync.dma_start(eye_sb[:], eye[:])
            load_at(0)
            load_xt(0)
            load_xt(1)
            nc.sync.dma_start(an_sb[:], an[:])
            for j in range(NN):
                load_xn(j)
            load_at(3)
            load_xt(2)
            load_xt(3)
            load_at(6)
            load_xt(4)
            load_xt(5)
            load_xt(6)
            load_xt(7)
            load_xt(8)
            load_xsp(0, 0)
            load_xsp(0, 1)
            load_xsp(0, 2)
            load_xsp(1, 0)
            load_xsp(1, 1)
            load_xsp(1, 2)

            # ---------------- compute ----------------
            ps1 = {(m, n): psp.tile([128, 512], F32, tag=f"g1m{m}n{n}",
                                    name=f"g1m{m}n{n}")
                   for m in range(MT) for n in range(NT)}

            order = _g1_order()

            def g1_step(idx):
                kind, s = order[idx]
                first = idx == 0
                last = idx == len(order) - 1
                for n in range(NT):
                    for m in range(MT):
                        if kind == "d":
                            nc.tensor.matmul(ps1[(m, n)][:], at_s(s, m),
                                             xt_s(s, n), start=first,
                                             stop=last, perf_mode=DR)
                        else:
                            nc.tensor.matmul(
                                ps1[(m, n)][:],
                                an_sb[:, s * MO + 128 * m:
                                      s * MO + 128 * (m + 1)],
                                xn_t[s][:, 512 * n:512 * (n + 1)],
                                start=first, stop=last)

            sp_ps = {}

            def sp_chunk(b, m2):
                # transposed spatial conv: stationary = x slices, moving = w.
                # 16 tiny DR matmuls land [h'-block, (w,co)] directly in the
                # spectral output layout.
                j = b * MT + m2
                ps = psw.tile([128, 512], F32, tag=f"spp{j % 2}",
                              name=f"spp{j}")
                sp_ps[(b, m2)] = ps
                xv = xsp_g[(b, m2)][:].rearrange("p (i n) -> p i n", i=2)
                for w in range(WS):
                    nc.tensor.matmul(ps[:, 32 * w:32 * (w + 1)],
                                     xv[:, :, w::WS], wsp_v,
                                     start=True, stop=True, perf_mode=DR)

            def sp_relu(b, m2):
                # unscale (1/32) + relu, psum -> f16 sbuf; ACT/DVE alternate
                dst = sp_sb[m2][:, 512 * b:512 * (b + 1)]
                if (b * MT + m2) % 2 == 0:
                    nc.scalar.activation(dst, sp_ps[(b, m2)][:], RELU,
                                         scale=1.0 / WSP_SCALE)
                else:
                    nc.vector.tensor_scalar(dst, sp_ps[(b, m2)][:],
                                            1.0 / WSP_SCALE, 0.0,
                                            AluOpType.mult, AluOpType.max)

            def relu_mn(m, n):
                # relu1 scaled x64 (G absorbs /64); only ACT/DVE read PSUM
                nsl = slice(512 * n, 512 * (n + 1))
                use_act = m == 0 or (m == 2 and n == 0)
                if use_act:
                    nc.scalar.activation(relu_t[m][:, nsl], ps1[(m, n)][:],
                                         RELU, bias=bspec_sb[:, m:m + 1])
                else:
                    nc.vector.tensor_scalar(relu_t[m][:, nsl], ps1[(m, n)][:],
                                            bspec_sb[:, m:m + 1], 0.0,
                                            AluOpType.add, AluOpType.max)

            ps2 = {}

            def g2_n(n):
                nsl = slice(512 * n, 512 * (n + 1))
                for m2 in range(MT):
                    ps2[(m2, n)] = psp.tile([128, 512], F32,
                                            tag=f"g1m{m2}n{n}", name="g2")
                for k in range(MT):
                    for m2 in range(MT):
                        nc.tensor.matmul(ps2[(m2, n)][:], gt_km(k, m2),
                                         relu_t[k][:, nsl],
                                         start=(k == 0), stop=False)

            def add_sp(m2, n):
                # out = spec + sp_relu, folded on the PE: identity matmul
                # accumulates sp_relu into ps2 and closes the group
                nsl = slice(512 * n, 512 * (n + 1))
                nc.tensor.matmul(ps2[(m2, n)][:], eye_sb[:],
                                 sp_sb[m2][:, nsl], start=False, stop=True)

            def copy_out(m2, n):
                # psum -> f16 sbuf; ACT/DVE alternate
                nsl = slice(512 * n, 512 * (n + 1))
                if (m2 + n) % 2 == 0:
                    nc.scalar.activation(out_sb[m2][:, nsl], ps2[(m2, n)][:],
                                         mybir.ActivationFunctionType.Copy)
                else:
                    nc.vector.tensor_copy(out_sb[m2][:, nsl],
                                          ps2[(m2, n)][:])

            def store(m2, n):
                nc.gpsimd.dma_start(
                    out_d[:, m2 * NCOL + 512 * n:m2 * NCOL + 512 * (n + 1)],
                    out_sb[m2][:, 512 * n:512 * (n + 1)])

            # ---- PE order ----
            for idx in range(len(order)):
                g1_step(idx)
            for n in range(NT):
                for m in range(MT):
                    relu_mn(m, n)
            sp_chunk(0, 0)
            sp_relu(0, 0)
            sp_chunk(0, 1)
            sp_relu(0, 1)
            g2_n(0)
            sp_chunk(0, 2)
            sp_relu(0, 2)
            add_sp(0, 0)
            add_sp(1, 0)
            add_sp(2, 0)
            copy_out(0, 0)
            copy_out(1, 0)
            copy_out(2, 0)
            store(0, 0)
            store(1, 0)
            store(2, 0)
            sp_chunk(1, 0)
            sp_relu(1, 0)
            g2_n(1)
            sp_chunk(1, 1)
            sp_relu(1, 1)
            sp_chunk(1, 2)
            sp_relu(1, 2)
            add_sp(0, 1)
            add_sp(1, 1)
            add_sp(2, 1)
            copy_out(0, 1)
            copy_out(1, 1)
            copy_out(2, 1)
            store(0, 1)
            store(1, 1)
            store(2, 1)

    nc.compile()
    _cache["main"] = nc
    return nc


def _host_prep(x, w_spatial, b_spatial, w_spectral, b_spectral):
    """Shared (weight) swizzles."""
    F, G = _dft_constants()
    keep_f, keep_o = _spec_keep_idx()

    A = w_spectral[keep_o][:, keep_f] @ F                    # [384, 2688]
    arrA = np.ascontiguousarray((A * AT_SCALE).T)            # [2688, 384]
    hcut = KD_DR * 256
    at_np = np.ascontiguousarray(
        arrA[:hcut].reshape(KD_DR * 2, 128, MO).transpose(1, 0, 2)
        .reshape(128, KD_DR * 2 * MO)).astype(E4M3)
    an_np = np.ascontiguousarray(
        arrA[hcut:].reshape(NN, 128, MO).transpose(1, 0, 2)
        .reshape(128, NN * MO)).astype(np.float16)
    gt_np = np.ascontiguousarray(
        (G.T / AT_SCALE).reshape(MT, 128, MO).transpose(1, 0, 2)
        .reshape(128, MT * MO)).astype(np.float16)
    bspec_np = np.ascontiguousarray(
        (b_spectral[keep_o] * AT_SCALE).reshape(MT, 128).T).astype(np.float32)
    wbase = (w_spatial[:, :, :, 0].transpose(1, 2, 0).reshape(C * 7, C)
             * WSP_SCALE)
    wsp_np = np.concatenate([
        wbase.reshape(2, KH, C).transpose(1, 0, 2).reshape(KH, 2 * C),
        np.concatenate([b_spatial * WSP_SCALE, np.zeros(C)])[None, :],
    ], axis=0).astype(E4M3)                                  # [113, 64]
    return at_np, an_np, gt_np, bspec_np, wsp_np


def kernel(x, w_spatial, b_spatial, w_spectral, b_spectral):
    x = np.ascontiguousarray(x, dtype=np.float32)
    w_spatial = np.asarray(w_spatial, dtype=np.float32)
    b_spatial = np.asarray(b_spatial, dtype=np.float32)
    w_spectral = np.asarray(w_spectral, dtype=np.float32)
    b_spectral = np.asarray(b_spectral, dtype=np.float32)

    at_np, an_np, gt_np, bspec_np, wsp_np = _host_prep(
        x, w_spatial, b_spatial, w_spectral, b_spectral)
    if "eye" not in _cache:
        _cache["eye"] = np.eye(128, dtype=np.float16)
    core_ids = list(range(N_CORES))
    hcut = KD_DR * 256

    in_maps = []
    for i in core_ids:
        xs = x[:, :, :, WS * i:WS * (i + 1)]                 # [B, C, H, WS]
        arr = xs.transpose(2, 0, 3, 1).reshape(H, NCOL)      # [H, (b,w,c)]
        xt_np = np.ascontiguousarray(
            arr[:hcut].reshape(KD_DR * 2, 128, NCOL).transpose(1, 0, 2)
            .reshape(128, KD_DR * 2 * NCOL)).astype(E4M3)
        xn_np = np.ascontiguousarray(
            arr[hcut:].reshape(NN, 128, NCOL).transpose(1, 0, 2)
            .reshape(128, NN * NCOL)).astype(E3M4)
        spbase = (xs.reshape(B, C, OUT_H, 7, WS).transpose(1, 3, 0, 2, 4)
                  .reshape(C * 7, NSP))                      # [(c,t),(b,h',w)]
        xsp_np = np.concatenate([
            spbase.reshape(2, KH, NSP).transpose(1, 0, 2).reshape(KH, 2 * NSP),
            np.concatenate([np.ones(NSP, np.float32),
                            np.zeros(NSP, np.float32)])[None, :],
        ], axis=0).astype(E4M3)                              # [113, 2*NSP]
        in_maps.append({"at": at_np, "an": an_np, "xt": xt_np, "xn": xn_np,
                        "gt": gt_np, "bspec": bspec_np, "wsp": wsp_np,
                        "xsp": xsp_np, "eye": _cache["eye"]})

    nc = _build_main()
    kw = {}
    if bool(int(os.environ.get("KERNEL_TRACE", "0"))):
        d = os.environ.get("KERNEL_TRACE_DIR", "/tmp/ktrace") + "/main"
        os.makedirs(d, exist_ok=True)
        kw = dict(trace=True, tmpdir=d)
    res = run_bass_kernel_spmd(nc, in_maps, core_ids, **kw)
    global LAST_EXEC_NS
    LAST_EXEC_NS = res.exec_time_ns

    # ---- host: unshard (spatial already added on-device) ----
    out = np.empty((B, C, OUT_H, W), np.float32)
    for i in core_ids:
        o = (res.results[i]["out"].astype(np.float32)
             .reshape(128, MT, B, WS, C).transpose(2, 4, 1, 0, 3)
             .reshape(B, C, OUT_H, WS))
        out[:, :, :, WS * i:WS * (i + 1)] = o
    return out


# revision 21
# speedup vs baseline: 1.4789x; 1.1369x over previous
"""Trainium2 Bass kernel for the FFTBlock problem (B=2, C=32, H=2688, W=128).

Math (reference):
  spatial  = relu(conv7x1_s7(x) + b_spatial)                        [B,C,384,W]
  spectral = irfft(relu(w_spectral @ rfft_concat(x) + b_spectral))  per (b,c,w)
  out = spatial + spectral

rfft/irfft along H are linear, so with F the real-ified rfft matrix and G the
irfft matrix (dead rows/cols dropped):
  spectral_col = G @ relu(A @ x_col + b),   A = w_spectral @ F  [384, 2688]

Device plan (W sharded 8 x 16 columns, one launch per core):
  GEMM1  conv[384, 1024] = A @ x_cols:
         - KD_DR DoubleRow steps (K=256, both operands e4m3, x64 A scale)
         - NN normal-mode k-tiles (K=128, f16 A x e3m4 x) for accuracy;
           they double as fat PE fillers mid-loop so the p-state never drops
  relu   (ACT/DVE/Pool, bias)            -> f16 (scaled x64; G absorbs /64)
  GEMM2  spec[384, 1024] = (G/64) @ relu  (f16, 3 k-tiles)
  spatial conv TRANSPOSED: stationary = x chunks [(c,t)=224+bias row,
         h'-block], moving = w_spatial (e4m3 x32, bias folded as an extra
         contraction row), DoubleRow -> psum [h', (b,w,co)] -- same layout as
         spec, so the final add happens on-device and only ONE output tensor
         is stored.
  out[m2,n] = ps2 + sp_relu  (DVE/Pool scalar_tensor_tensor), stored f16.

Column order everywhere is (b, w, c) so the spatial conv's 32-channel output
blocks are contiguous in the spectral column space. All DRAM layouts are
pre-swizzled on host to partition-major so every DMA moves >=512B runs.
"""

import os

import numpy as np
import ml_dtypes

import concourse.bacc as bacc
import concourse.mybir as mybir
import concourse.tile as tile
from concourse.bass_utils import run_bass_kernel_spmd
from concourse.alu_op_type import AluOpType

N_CORES = 8
B, C, H, W = 2, 32, 2688, 128
FREQ_IN = H // 2 + 1            # 1345
OUT_H = 384
FREQ_OUT = OUT_H // 2 + 1       # 193
MO = 2 * FREQ_OUT - 2           # 384 usable conv channels
WS = W // N_CORES               # 16 width columns per core
NCOL = B * WS * C               # 1024 spectral columns per core, (b, w, c)
NSP = B * OUT_H * WS            # 12288 spatial cols (b, h', w)
OLDK = H // 128                 # 21 k-tiles of 128
KD_DR = 9                       # DoubleRow steps (cover old-k 0..2*KD_DR-1)
NN = OLDK - 2 * KD_DR           # normal-mode k-tiles (incl. the odd tail)
MT = 3                          # 128-row m-tiles (G1 out / G2 out)
NT = 2                          # 512-col n halves; n == b
KH = 112                        # (c,t) DR half-pairs for spatial
KHB = KH + 1                    # +1 bias row

AT_SCALE = 64.0                 # fp8 range helper for A = w_spec @ F
WSP_SCALE = 32.0                # fp8 range helper for the tiny spatial weights

F32 = mybir.dt.float32
F16 = mybir.dt.float16
F8E4 = mybir.dt.float8e4
F8E3 = mybir.dt.float8e3
RELU = mybir.ActivationFunctionType.Relu
DR = mybir.MatmulPerfMode.DoubleRow
E4M3 = ml_dtypes.float8_e4m3
E3M4 = ml_dtypes.float8_e3m4

_cache = {}
LAST_EXEC_NS = None


def _dft_constants():
    """F [2688, 2688] (rfft, ortho, dead rows dropped) and G [384, 384]
    (irfft, ortho, dead cols dropped)."""
    if "F" in _cache:
        return _cache["F"], _cache["G"]
    Fc = np.fft.rfft(np.eye(H), axis=0, norm="ortho")       # [1345, 2688]
    F = np.concatenate([Fc.real, Fc.imag[1:FREQ_IN - 1]], axis=0)
    F = np.ascontiguousarray(F, dtype=np.float32)           # [2688, 2688]
    G_re = np.fft.irfft(np.eye(FREQ_OUT), n=OUT_H, axis=0, norm="ortho")
    G_im = np.fft.irfft(1j * np.eye(FREQ_OUT), n=OUT_H, axis=0, norm="ortho")
    G = np.concatenate([G_re, G_im[:, 1:FREQ_OUT - 1]], axis=1)
    G = np.ascontiguousarray(G, dtype=np.float32)           # [384, 384]
    _cache["F"] = F
    _cache["G"] = G
    return F, G


def _spec_keep_idx():
    keep_f = list(range(FREQ_IN)) + [FREQ_IN + k for k in range(1, FREQ_IN - 1)]
    keep_o = list(range(FREQ_OUT)) + [FREQ_OUT + k for k in range(1, FREQ_OUT - 1)]
    return np.array(keep_f), np.array(keep_o)


def _g1_order():
    """PE-side k-step order: DR steps with the fat normal tiles interleaved
    early-mid so the PE tracks the DMA stream without stalling."""
    order = [("d", s) for s in range(KD_DR)]
    pos = 4
    for j in range(NN):
        order.insert(min(pos, len(order)), ("n", j))
        pos += 2
    return order


def _build_main():
    if "main" in _cache:
        return _cache["main"]
    nc = bacc.Bacc("TRN2", target_bir_lowering=False, debug=False,
                   num_devices=N_CORES)
    at = nc.dram_tensor("at", [128, KD_DR * 2 * MO], F8E4,
                        kind="ExternalInput").ap()
    an = nc.dram_tensor("an", [128, NN * MO], F16, kind="ExternalInput").ap()
    xt = nc.dram_tensor("xt", [128, KD_DR * 2 * NCOL], F8E4,
                        kind="ExternalInput").ap()
    xn = nc.dram_tensor("xn", [128, NN * NCOL], F8E3,
                        kind="ExternalInput").ap()
    gt = nc.dram_tensor("gt", [128, MT * MO], F16, kind="ExternalInput").ap()
    bspec = nc.dram_tensor("bspec", [128, MT], F32, kind="ExternalInput").ap()
    wsp = nc.dram_tensor("wsp", [KHB, 2 * C], F8E4, kind="ExternalInput").ap()
    xsp = nc.dram_tensor("xsp", [KHB, 2 * NSP], F8E4,
                         kind="ExternalInput").ap()
    out_d = nc.dram_tensor("out", [128, MT * NCOL], F16,
                           kind="ExternalOutput").ap()

    with tile.TileContext(nc) as tc:
        with tc.tile_pool(name="const", bufs=1) as cst, \
             tc.tile_pool(name="atp", bufs=1) as atp, \
             tc.tile_pool(name="xtp", bufs=1) as xtp, \
             tc.tile_pool(name="xspp", bufs=1) as xspp, \
             tc.tile_pool(name="relu", bufs=1) as rlp, \
             tc.tile_pool(name="outp", bufs=1) as outp, \
             tc.tile_pool(name="ps", bufs=1, space="PSUM") as psp, \
             tc.tile_pool(name="psw", bufs=1, space="PSUM") as psw:

            # ---------------- SBUF tiles ----------------
            AGRP = [(0, 3), (3, 6), (6, KD_DR)]
            at_g = {}
            for g0, g1 in AGRP:
                at_g[g0] = atp.tile([128, (g1 - g0) * 2 * MO], F8E4,
                                    tag=f"at{g0}", name=f"at{g0}")
            an_sb = atp.tile([128, NN * MO], F16, tag="an", name="an")
            xt_t = [xtp.tile([128, 2 * NCOL], F8E4, tag=f"xt{s}",
                             name=f"xt{s}") for s in range(KD_DR)]
            xn_t = [xtp.tile([128, NCOL], F8E3, tag=f"xn{j}",
                             name=f"xn{j}") for j in range(NN)]
            gt_sb = cst.tile([128, MT * MO], F16, tag="gt", name="gt")
            bspec_sb = cst.tile([128, MT], F32, tag="bspec", name="bspec")
            wsp_sb = cst.tile([KHB, 2 * C], F8E4, tag="wsp", name="wsp")
            xsp_g = {(b, m2): xspp.tile([KHB, 2 * 2048], F8E4,
                                        tag=f"xsp{b}{m2}", name=f"xsp{b}{m2}")
                     for b in range(B) for m2 in range(MT)}
            relu_t = [rlp.tile([128, NCOL], F16, tag=f"relu{m}",
                               name=f"relu{m}") for m in range(MT)]
            sp_sb = [rlp.tile([128, NCOL], F16, tag=f"sp{m2}",
                              name=f"sp{m2}") for m2 in range(MT)]
            out_sb = [outp.tile([128, NCOL], F16, tag=f"o{m2}",
                                name=f"o{m2}") for m2 in range(MT)]

            def at_s(s, m):
                """DR stationary [128, 2, 128] for DR step s, m-tile m."""
                g0 = max(g for g, _ in AGRP if g <= s)
                off = (s - g0) * 2 * MO
                v = at_g[g0][:, off:off + 2 * MO].rearrange(
                    "p (i m) -> p i m", i=2)
                return v[:, :, 128 * m:128 * (m + 1)]

            def xt_s(s, n):
                v = xt_t[s][:].rearrange("p (i n) -> p i n", i=2)
                return v[:, :, 512 * n:512 * (n + 1)]

            wsp_v = wsp_sb[:].rearrange("p (i m) -> p i m", i=2)
            gt_km = lambda k, m2: gt_sb[:, k * MO + 128 * m2:
                                        k * MO + 128 * (m2 + 1)]

            # ---------------- DMA emission (sync queue, in order) --------
            def load_at(g0):
                g1 = dict(AGRP)[g0]
                nc.sync.dma_start(at_g[g0][:],
                                  at[:, g0 * 2 * MO:g1 * 2 * MO])

            def load_xt(s):
                nc.sync.dma_start(xt_t[s][:],
                                  xt[:, s * 2 * NCOL:(s + 1) * 2 * NCOL])

            def load_xn(j):
                nc.sync.dma_start(xn_t[j][:],
                                  xn[:, j * NCOL:(j + 1) * NCOL])

            def load_xsp(b, m2):
                src = xsp.rearrange("p (i n) -> p i n", i=2)[
                    :, :, b * (OUT_H * WS) + m2 * 2048:
                    b * (OUT_H * WS) + (m2 + 1) * 2048]
                dst = xsp_g[(b, m2)][:].rearrange("p (i n) -> p i n", i=2)
                nc.sync.dma_start(dst, src)

            # order tuned so the PE (starting at xt0+900ns sem prop) never
            # starves; small consts hide mid-stream behind big transfers; the
            # last input (xsp b1 m2=2) has the shortest dependent chain
            load_at(0)
            load_xt(0)
            load_xt(1)
            nc.sync.dma_start(gt_sb[:], gt[:])
            nc.sync.dma_start(bspec_sb[:], bspec[:])
            nc.sync.dma_start(wsp_sb[:], wsp[:])
            load_xt(2)
            load_at(3)
            load_xt(3)
            load_xt(4)
            nc.sync.dma_start(an_sb[:], an[:])
            for j in range(NN):
                load_xn(j)
            load_at(6)
            load_xt(5)
            load_xt(6)
            load_xt(7)
            load_xt(8)
            load_xsp(0, 0)
            load_xsp(0, 1)
            load_xsp(0, 2)
            load_xsp(1, 0)
            load_xsp(1, 1)
            load_xsp(1, 2)

            # ---------------- compute ----------------
            # ps1[m] spans both n halves (2 PSUM banks) so relu runs one
            # full-width op per m
            ps1 = {m: psp.tile([128, 1024], F32, tag=f"g1m{m}",
                               name=f"g1m{m}") for m in range(MT)}

            order = _g1_order()

            def g1_step(idx):
                kind, s = order[idx]
                first = idx == 0
                last = idx == len(order) - 1
                for n in range(NT):
                    nsl = slice(512 * n, 512 * (n + 1))
                    for m in range(MT):
                        if kind == "d":
                            nc.tensor.matmul(ps1[m][:, nsl], at_s(s, m),
                                             xt_s(s, n), start=first,
                                             stop=last, perf_mode=DR)
                        else:
                            nc.tensor.matmul(
                                ps1[m][:, nsl],
                                an_sb[:, s * MO + 128 * m:
                                      s * MO + 128 * (m + 1)],
                                xn_t[s][:, 512 * n:512 * (n + 1)],
                                start=first, stop=last)

            sp_ps = {}

            def sp_chunk(b, m2):
                # transposed spatial conv: stationary = x slices, moving = w.
                # 16 tiny DR matmuls land [h'-block, (w,co)] directly in the
                # spectral output layout.
                j = b * MT + m2
                ps = psw.tile([128, 512], F32, tag=f"spp{j % 2}",
                              name=f"spp{j}")
                sp_ps[(b, m2)] = ps
                xv = xsp_g[(b, m2)][:].rearrange("p (i n) -> p i n", i=2)
                for w in range(WS):
                    nc.tensor.matmul(ps[:, 32 * w:32 * (w + 1)],
                                     xv[:, :, w::WS], wsp_v,
                                     start=True, stop=True, perf_mode=DR)

            def sp_relu(b, m2):
                # unscale (1/32) + relu, psum -> f16 sbuf, on ACT (DVE is
                # loaded with the fused output adds)
                dst = sp_sb[m2][:, 512 * b:512 * (b + 1)]
                nc.scalar.activation(dst, sp_ps[(b, m2)][:], RELU,
                                     scale=1.0 / WSP_SCALE)

            def relu_m(m, half=None):
                # relu1 scaled x64 (G absorbs /64); only ACT/DVE read PSUM.
                # m=0/1 full-width on ACT/DVE; m=2 split across both.
                nsl = slice(0, NCOL) if half is None else \
                    slice(512 * half, 512 * (half + 1))
                use_act = m == 0 or (m == 2 and half == 0)
                if use_act:
                    nc.scalar.activation(relu_t[m][:, nsl], ps1[m][:, nsl],
                                         RELU, bias=bspec_sb[:, m:m + 1])
                else:
                    nc.vector.tensor_scalar(relu_t[m][:, nsl], ps1[m][:, nsl],
                                            bspec_sb[:, m:m + 1], 0.0,
                                            AluOpType.add, AluOpType.max)

            ps2 = {}

            def g2_n(n):
                nsl = slice(512 * n, 512 * (n + 1))
                if not ps2:
                    for m2 in range(MT):
                        ps2[m2] = psp.tile([128, 1024], F32,
                                           tag=f"g1m{m2}", name=f"g2m{m2}")
                for k in range(MT):
                    for m2 in range(MT):
                        nc.tensor.matmul(ps2[m2][:, nsl], gt_km(k, m2),
                                         relu_t[k][:, nsl],
                                         start=(k == 0), stop=(k == MT - 1))

            def fadd(m2, n):
                # out = spec(psum) + sp_relu(sbuf), fused on DVE
                nsl = slice(512 * n, 512 * (n + 1))
                nc.vector.scalar_tensor_tensor(
                    out_sb[m2][:, nsl], ps2[m2][:, nsl], 1.0,
                    sp_sb[m2][:, nsl], AluOpType.mult, AluOpType.add)

            def store(m2, n):
                nc.sync.dma_start(
                    out_d[:, m2 * NCOL + 512 * n:m2 * NCOL + 512 * (n + 1)],
                    out_sb[m2][:, 512 * n:512 * (n + 1)])

            # ---- PE order ----
            for idx in range(len(order)):
                g1_step(idx)
            relu_m(0)
            relu_m(1)
            relu_m(2, 0)
            relu_m(2, 1)
            sp_chunk(0, 0)
            sp_relu(0, 0)
            sp_chunk(0, 1)
            sp_relu(0, 1)
            g2_n(0)
            sp_chunk(0, 2)
            sp_relu(0, 2)
            fadd(0, 0)
            fadd(1, 0)
            fadd(2, 0)
            sp_chunk(1, 0)
            sp_relu(1, 0)
            g2_n(1)
            sp_chunk(1, 1)
            sp_relu(1, 1)
            sp_chunk(1, 2)
            sp_relu(1, 2)
            fadd(0, 1)
            fadd(1, 1)
            fadd(2, 1)
            store(0, 0)
            store(1, 0)
            store(2, 0)
            store(0, 1)
            store(1, 1)
            store(2, 1)

    nc.compile()
    _cache["main"] = nc
    return nc


def _host_prep(x, w_spatial, b_spatial, w_spectral, b_spectral):
    """Shared (weight) swizzles."""
    F, G = _dft_constants()
    keep_f, keep_o = _spec_keep_idx()

    A = w_spectral[keep_o][:, keep_f] @ F                    # [384, 2688]
    arrA = np.ascontiguousarray((A * AT_SCALE).T)            # [2688, 384]
    hcut = KD_DR * 256
    at_np = np.ascontiguousarray(
        arrA[:hcut].reshape(KD_DR * 2, 128, MO).transpose(1, 0, 2)
        .reshape(128, KD_DR * 2 * MO)).astype(E4M3)
    an_np = np.ascontiguousarray(
        arrA[hcut:].reshape(NN, 128, MO).transpose(1, 0, 2)
        .reshape(128, NN * MO)).astype(np.float16)
    gt_np = np.ascontiguousarray(
        (G.T / AT_SCALE).reshape(MT, 128, MO).transpose(1, 0, 2)
        .reshape(128, MT * MO)).astype(np.float16)
    bspec_np = np.ascontiguousarray(
        (b_spectral[keep_o] * AT_SCALE).reshape(MT, 128).T).astype(np.float32)
    wbase = (w_spatial[:, :, :, 0].transpose(1, 2, 0).reshape(C * 7, C)
             * WSP_SCALE)
    wsp_np = np.concatenate([
        wbase.reshape(2, KH, C).transpose(1, 0, 2).reshape(KH, 2 * C),
        np.concatenate([b_spatial * WSP_SCALE, np.zeros(C)])[None, :],
    ], axis=0).astype(E4M3)                                  # [113, 64]
    return at_np, an_np, gt_np, bspec_np, wsp_np


def kernel(x, w_spatial, b_spatial, w_spectral, b_spectral):
    x = np.ascontiguousarray(x, dtype=np.float32)
    w_spatial = np.asarray(w_spatial, dtype=np.float32)
    b_spatial = np.asarray(b_spatial, dtype=np.float32)
    w_spectral = np.asarray(w_spectral, dtype=np.float32)
    b_spectral = np.asarray(b_spectral, dtype=np.float32)

    at_np, an_np, gt_np, bspec_np, wsp_np = _host_prep(
        x, w_spatial, b_spatial, w_spectral, b_spectral)
    core_ids = list(range(N_CORES))
    hcut = KD_DR * 256

    in_maps = []
    for i in core_ids:
        xs = x[:, :, :, WS * i:WS * (i + 1)]                 # [B, C, H, WS]
        arr = xs.transpose(2, 0, 3, 1).reshape(H, NCOL)      # [H, (b,w,c)]
        xt_np = np.ascontiguousarray(
            arr[:hcut].reshape(KD_DR * 2, 128, NCOL).transpose(1, 0, 2)
            .reshape(128, KD_DR * 2 * NCOL)).astype(E4M3)
        xn_np = np.ascontiguousarray(
            arr[hcut:].reshape(NN, 128, NCOL).transpose(1, 0, 2)
            .reshape(128, NN * NCOL)).astype(E3M4)
        spbase = (xs.reshape(B, C, OUT_H, 7, WS).transpose(1, 3, 0, 2, 4)
                  .reshape(C * 7, NSP))                      # [(c,t),(b,h',w)]
        xsp_np = np.concatenate([
            spbase.reshape(2, KH, NSP).transpose(1, 0, 2).reshape(KH, 2 * NSP),
            np.concatenate([np.ones(NSP, np.float32),
                            np.zeros(NSP, np.float32)])[None, :],
        ], axis=0).astype(E4M3)                              # [113, 2*NSP]
        in_maps.append({"at": at_np, "an": an_np, "xt": xt_np, "xn": xn_np,
                        "gt": gt_np, "bspec": bspec_np, "wsp": wsp_np,
                        "xsp": xsp_np})

    nc = _build_main()
    kw = {}
    if bool(int(os.environ.get("KERNEL_TRACE", "0"))):
        d = os.environ.get("KERNEL_TRACE_DIR", "/tmp/ktrace") + "/main"
        os.makedirs(d, exist_ok=True)
        kw = dict(trace=True, tmpdir=d)
    res = run_bass_kernel_spmd(nc, in_maps, core_ids, **kw)
    global LAST_EXEC_NS
    LAST_EXEC_NS = res.exec_time_ns

    # ---- host: unshard (spatial already added on-device) ----
    out = np.empty((B, C, OUT_H, W), np.float32)
    for i in core_ids:
        o = (res.results[i]["out"].astype(np.float32)
             .reshape(128, MT, B, WS, C).transpose(2, 4, 1, 0, 3)
             .reshape(B, C, OUT_H, WS))
        out[:, :, :, WS * i:WS * (i + 1)] = o
    return out
